# revision 23
# baseline (speedup 1.0000x reference)
"""Adaptive embedding lookup (nn.AdaptiveEmbedding) on 8 TRN2 NeuronCores.

Strategy (data-parallel over tokens, tables replicated, no collectives):

Host:
  - Bucket the 16384 tokens by embedding cluster, deal each bucket
    round-robin to the 8 cores, pad per-core buckets to a multiple of 128.
  - Clusters 0/1 (d_emb 1024/256) are FOLDED ON HOST: table' = emb @ proj.T
    * sqrt(d_proj) in f32, stored bf16 [20000, 1024].  On device these
    clusters are a pure gather -> output DMA (no matmul, no projection).
  - Cluster 2: table cast bf16 and zero-padded to 128 cols (256B rows).
  - Cluster 3: 8 rows packed per 128-col super-row; a host-shipped 0/1 mask
    selects the right 16-feature block per token before the matmul against
    the 8x-replicated projection.

Device (SPMD, identical graph on all 8 cores, one TileContext):
  - All gathers use gpsimd.indirect_dma_start (InstDMACopy + dynamic AP,
    int32 row offsets).  Unlike dma_gather this needs NO Q7 library load
    (which costs ~10us of Pool-engine stall), no wrapped int16 index
    layout, and no 32767-row table limit.
  - A burst of real warmup matmuls engages the PE HAM clock ramp while the
    gathers are in flight (PE-transposes don't count as HAM activity).
  - c0/c1: gathered rows stream straight out as output rows.
  - c2/c3: per 128-token group: PE-transpose the gathered rows (c3: after
    the mask multiply), then 2 matmuls (N=512) -> PSUM -> bf16 copy ->
    batched output DMA per unit.

Host: inverse-permute the 8 per-core outputs into [8, 2048, 1024] f32.
"""

import numpy as np
import ml_dtypes

import concourse.bacc as bacc
import concourse.bass as bass
import concourse.mybir as mybir
from concourse.bass_utils import run_bass_kernel_spmd
from concourse.tile import TileContext

N_TOKEN = 267735
D_PROJ = 1024
CUTOFF_ENDS = [0, 20000, 40000, 200000, 267735]
EMB_SCALE = float(D_PROJ) ** 0.5
N_CORES = 8
P = 128
NFREE = 512          # psum free-dim per matmul
C3_PACK = 8          # cluster-3 rows packed per super-row
C3_SROWS = -(-(CUTOFF_ENDS[4] - CUTOFF_ENDS[3]) // C3_PACK)  # 8467

BF16 = ml_dtypes.bfloat16

# Test-harness knobs (the grader never touches these).
TRACE = False
TRACE_CORES = None
LAST = {}
NWARM = 20

_GRAPH_CACHE = {}

UNIT_KEYS = [2, 3, 0, 1]   # gather/dispatch order: PE-feeding units first


def _indirect_gather_q(eng, out, in_, off_col, queue_name):
    """indirect_dma_start (gather form) with a selectable SWDGE queue.

    bass's indirect_dma_start pins queue="qPoolDynamic"; emitting the same
    InstDMACopy on qPoolDynamic{1,2,3} lets the Q7 descriptor generation for
    independent gathers overlap 4-wide (~1.1us each otherwise serializes).
    """
    out_ap = eng.lower_ap_dma(out, for_indirect_dma=True)
    in_ap = eng.lower_ap_dma(in_, for_indirect_dma=True)
    assert len(in_ap) == 1 and len(out_ap) == 1
    offset_ap = eng.lower_ap_dma(off_col)
    assert len(offset_ap) == 1
    in_ap.append(offset_ap[0])
    coef = in_.shape[1]
    in_ap[0].dynamic_ap_info = mybir.DynamicAccessPatternInfo(
        c=0,
        actual_ap=out.ap,
        indirect_dim_max_index=in_.shape[0],
        offset_expr=[
            mybir.DynamicAccessPatternOffsetExpr(
                coef=coef,
                aff_expr=mybir.DynamicAccessPatternOffsetExprAffExpr(
                    kind="IndirectArgId", arg_id=1,
                ),
            )
        ],
    )
    return eng.add_instruction(
        mybir.InstDMACopy(
            name=eng.bass.get_next_instruction_name(),
            queue=queue_name,
            mode="Copy",
            ins=in_ap,
            outs=out_ap,
            oob_is_err=True,
            cce_op=mybir.AluOpType.bypass,
        )
    )


def _build_graph(Ks, cnts):
    """Ks: unit -> group count; cnts: unit -> max used rows across cores."""
    key = tuple((Ks[u], cnts[u]) for u in UNIT_KEYS)
    if key in _GRAPH_CACHE:
        return _GRAPH_CACHE[key]

    K0, K1, K2, K3 = Ks[0], Ks[1], Ks[2], Ks[3]
    G = sum(Ks.values())
    NAUX = P + max(K3, 1) * P        # [identity | c3 sub-row select mask]

    nc = bacc.Bacc("TRN2", debug=False, num_swdge_queues=4,
                   enable_partition_id=False)
    off_ext = nc.declare_dram_parameter("off32", [P, G], mybir.dt.int32, False)
    aux_ext = nc.declare_dram_parameter("aux", [P, NAUX], mybir.dt.bfloat16, False)
    emb0_ext = nc.declare_dram_parameter("emb0p", [20000, 1024], mybir.dt.bfloat16, False)
    emb1_ext = nc.declare_dram_parameter("emb1p", [20000, 1024], mybir.dt.bfloat16, False)
    emb2_ext = nc.declare_dram_parameter("emb2z", [160000, 128], mybir.dt.bfloat16, False)
    emb3_ext = nc.declare_dram_parameter("emb3p", [C3_SROWS, 128], mybir.dt.bfloat16, False)
    pt2_ext = nc.declare_dram_parameter("pt2", [64, D_PROJ], mybir.dt.bfloat16, False)
    pt3_ext = nc.declare_dram_parameter("pt3s", [P, D_PROJ], mybir.dt.bfloat16, False)
    out_ext = nc.declare_dram_parameter("out", [G * P, D_PROJ], mybir.dt.bfloat16, True)

    gbase = {}
    acc = 0
    for u in UNIT_KEYS:
        gbase[u] = acc
        acc += Ks[u]

    with TileContext(nc) as tc:
        with tc.tile_pool(name="const", bufs=1) as constp, \
             tc.tile_pool(name="ps_o", bufs=5, space="PSUM") as psump, \
             tc.tile_pool(name="ps_t", bufs=2, space="PSUM") as psumtr, \
             tc.tile_pool(name="ps_w", bufs=1, space="PSUM") as psumw:
            off_sb = constp.tile([P, G], mybir.dt.int32, tag="off")
            nc.sync.dma_start(out=off_sb[:], in_=off_ext[:])
            aux_sb = constp.tile([P, NAUX], mybir.dt.bfloat16, tag="aux")
            nc.scalar.dma_start(out=aux_sb[:], in_=aux_ext[:])
            ident = aux_sb[:, 0:P]
            pt2_sb = constp.tile([64, D_PROJ], mybir.dt.bfloat16, tag="pt2")
            nc.scalar.dma_start(out=pt2_sb[:], in_=pt2_ext[:])
            pt3_sb = constp.tile([P, D_PROJ], mybir.dt.bfloat16, tag="pt3")
            nc.scalar.dma_start(out=pt3_sb[:], in_=pt3_ext[:])

            junk = constp.tile([P, NFREE], mybir.dt.bfloat16, tag="junk")
            nc.vector.memset(junk[:], 0.0)

            e0 = constp.tile([P, max(K0, 1), 1024], mybir.dt.bfloat16, tag="e0")
            e1 = constp.tile([P, max(K1, 1), 1024], mybir.dt.bfloat16, tag="e1")
            e2 = constp.tile([P, max(K2, 1), P], mybir.dt.bfloat16, tag="e2")
            e3 = constp.tile([P, max(K3, 1), P], mybir.dt.bfloat16, tag="e3")
            em3 = constp.tile([P, max(K3, 1), P], mybir.dt.bfloat16, tag="em3")
            eT2 = constp.tile([64, max(K2, 1) * P], mybir.dt.bfloat16, tag="eT2")
            eT3 = constp.tile([P, max(K3, 1) * P], mybir.dt.bfloat16, tag="eT3")
            osb2 = constp.tile([P, max(K2, 1), 1024], mybir.dt.bfloat16, tag="osb2")
            osb3 = constp.tile([P, max(K3, 1), 1024], mybir.dt.bfloat16, tag="osb3")

            # ---- gathers: one indirect DMA per 128-token group (the HW
            # ucode only handles a single offset column per instruction),
            # PE-feeding units first, spread across the 4 SWDGE queues
            qnames = ["qPoolDynamic", "qPoolDynamic1",
                      "qPoolDynamic2", "qPoolDynamic3"]
            gq = [0]
            for u in UNIT_KEYS:
                n = Ks[u]
                if n == 0:
                    continue
                dst, tab = {
                    0: (e0, emb0_ext), 1: (e1, emb1_ext),
                    2: (e2, emb2_ext), 3: (e3, emb3_ext),
                }[u]
                for j in range(n):
                    g = gbase[u] + j
                    _indirect_gather_q(
                        nc.gpsimd, dst[:, j, :], tab[:],
                        off_sb[:, g:g + 1], qnames[gq[0] % 4],
                    )
                    gq[0] += 1

            # ---- PE warmup: real matmuls engage the HAM clock ramp while
            # the gathers are in flight (transposes would not)
            for w in range(NWARM):
                wps = psumw.tile([P, NFREE], mybir.dt.float32, tag="wps")
                nc.tensor.matmul(
                    out=wps[:], lhsT=junk[:, 0:P], rhs=junk[:],
                    start=True, stop=True,
                )

            # ---- output DMA helper
            dma_engs = [nc.sync, nc.scalar]
            qsel = [0]

            def emit_out(src3d, g0, gb, cnt):
                F = cnt // P
                rpart = cnt - F * P
                eng = dma_engs[qsel[0] % 2]
                qsel[0] += 1
                if F > 0:
                    dst = out_ext[gb * P:(gb + F) * P, :].rearrange(
                        "(g p) e -> p g e", p=P
                    )
                    eng.dma_start(out=dst, in_=src3d[:, g0:g0 + F, :])
                if rpart > 0:
                    eng2 = dma_engs[qsel[0] % 2]
                    qsel[0] += 1
                    eng2.dma_start(
                        out=out_ext[(gb + F) * P:(gb + F) * P + rpart, :],
                        in_=src3d[0:rpart, g0 + F:g0 + F + 1, :].rearrange(
                            "p o e -> p (o e)"
                        ),
                    )

            # ---- c0/c1: gathered rows ARE the output rows
            if K0 > 0:
                emit_out(e0, 0, gbase[0], cnts[0])
            if K1 > 0:
                emit_out(e1, 0, gbase[1], cnts[1])

            # ---- c2: transpose each group, matmul, batched output DMA
            def mm_group(lhsT, pt, psums):
                for oc, ps in enumerate(psums):
                    nc.tensor.matmul(
                        out=ps[:], lhsT=lhsT,
                        rhs=pt[:, oc * NFREE:(oc + 1) * NFREE],
                        start=True, stop=True,
                    )

            def do_unit(u, n, e_in, eT, eTp, d, pt, osb):
                for j in range(n):
                    tr = psumtr.tile([P, P], mybir.dt.bfloat16, tag="tr")
                    nc.tensor.transpose(
                        out=tr[:], in_=e_in[:, j, :], identity=ident
                    )
                    nc.any.tensor_copy(
                        out=eT[:, j * P:(j + 1) * P], in_=tr[0:eTp, :]
                    )
                    ps0 = psump.tile([P, NFREE], mybir.dt.float32, tag="ps")
                    ps1 = psump.tile([P, NFREE], mybir.dt.float32, tag="ps")
                    mm_group(eT[0:d, j * P:(j + 1) * P], pt, (ps0, ps1))
                    for oc, ps in enumerate((ps0, ps1)):
                        nc.any.tensor_copy(
                            out=osb[:, j, oc * NFREE:(oc + 1) * NFREE],
                            in_=ps[:],
                        )
                rows = cnts[u]
                emit_out(osb, 0, gbase[u],
                         n * P if n * P - rows <= 16 else rows)

            if K2 > 0:
                do_unit(2, K2, e2, eT2, 64, 64, pt2_sb, osb2)

            # ---- c3: mask-select, then transpose + matmul
            if K3 > 0:
                mask_view = aux_sb[:, P:P + K3 * P].rearrange(
                    "p (j i) -> p j i", j=K3
                )
                nc.vector.tensor_tensor(
                    out=em3[:, 0:K3, :], in0=e3[:, 0:K3, :], in1=mask_view,
                    op=mybir.AluOpType.mult,
                )
                do_unit(3, K3, em3, eT3, P, P, pt3_sb, osb3)

    nc.compile()
    _GRAPH_CACHE[key] = nc
    return nc


def kernel(inp, emb0, emb1, emb2, emb3, proj0, proj1, proj2, proj3):
    inp = np.asarray(inp)
    embs = [np.asarray(e, dtype=np.float32) for e in (emb0, emb1, emb2, emb3)]
    projs = [np.asarray(p, dtype=np.float32) for p in (proj0, proj1, proj2, proj3)]
    B, S = inp.shape
    flat = inp.reshape(-1).astype(np.int64)
    T = flat.shape[0]

    # ---- host-side bucketing -------------------------------------------
    flat = np.clip(flat, 0, N_TOKEN - 1)
    cluster = np.clip(
        np.searchsorted(np.asarray(CUTOFF_ENDS[1:]), flat, side="right"), 0, 3
    )
    local = flat - np.asarray(CUTOFF_ENDS)[cluster]

    unit_pos = {u: np.nonzero(cluster == u)[0] for u in UNIT_KEYS}
    core_lists = {u: [unit_pos[u][k::N_CORES] for k in range(N_CORES)]
                  for u in UNIT_KEYS}
    cnts = {u: max(len(core_lists[u][k]) for k in range(N_CORES))
            for u in UNIT_KEYS}
    Ks = {u: int(-(-max(cnts[u], 1) // P)) for u in UNIT_KEYS}
    G = sum(Ks.values())
    K3 = Ks[3]

    gbase = {}
    acc = 0
    for u in UNIT_KEYS:
        gbase[u] = acc
        acc += Ks[u]

    NAUX = P + max(K3, 1) * P
    blkid = np.arange(128) // 16

    off_maps, aux_maps, row_maps = [], [], []
    for k in range(N_CORES):
        off = np.zeros((P, G), dtype=np.int32)
        row_map = np.full(G * P, -1, dtype=np.int64)
        aux = np.zeros((P, NAUX), dtype=np.float32)
        aux[:, 0:P] = np.eye(P, dtype=np.float32)
        for u in UNIT_KEYS:
            n = Ks[u]
            if n == 0:
                continue
            lst = core_lists[u][k]
            lv = local[lst]
            vals = (lv // C3_PACK) if u == 3 else lv
            m = np.arange(len(lst))
            off[m % P, gbase[u] + m // P] = vals
            row_map[(gbase[u] + m // P) * P + (m % P)] = lst
            if u == 3:
                s_arr = np.zeros((P, max(K3, 1)), dtype=np.int64)
                s_arr[m % P, m // P] = lv % C3_PACK
                mask = (blkid[None, None, :] == s_arr[:, :, None])
                aux[:, P:P + K3 * P] = mask.reshape(P, K3 * P)
        off_maps.append(np.ascontiguousarray(off))
        aux_maps.append(np.ascontiguousarray(aux.astype(BF16)))
        row_maps.append(row_map)

    # ---- table/projection prep -----------------------------------------
    emb0p = np.ascontiguousarray(
        (embs[0] @ (projs[0].T * EMB_SCALE)).astype(BF16))
    emb1p = np.ascontiguousarray(
        (embs[1] @ (projs[1].T * EMB_SCALE)).astype(BF16))
    emb2z = np.zeros((160000, 128), dtype=BF16)
    emb2z[:, :64] = embs[2].astype(BF16)
    emb2z = np.ascontiguousarray(emb2z)
    e3flat = embs[3]
    pad3 = C3_SROWS * C3_PACK - e3flat.shape[0]
    e3flat = np.concatenate([e3flat, np.zeros((pad3, 16), np.float32)], axis=0)
    emb3p = np.ascontiguousarray(e3flat.reshape(C3_SROWS, 128).astype(BF16))

    pt2 = np.ascontiguousarray((projs[2].T * EMB_SCALE).astype(BF16))
    pt3 = projs[3].T * EMB_SCALE
    pt3s = np.ascontiguousarray(np.tile(pt3, (C3_PACK, 1)).astype(BF16))

    in_maps = []
    for k in range(N_CORES):
        in_maps.append({
            "off32": off_maps[k], "aux": aux_maps[k],
            "emb0p": emb0p, "emb1p": emb1p, "emb2z": emb2z, "emb3p": emb3p,
            "pt2": pt2, "pt3s": pt3s,
        })

    # ---- device --------------------------------------------------------
    nc = _build_graph(Ks, cnts)
    res = run_bass_kernel_spmd(
        nc,
        in_maps,
        core_ids=list(range(N_CORES)),
        trace=TRACE,
        trace_cores=TRACE_CORES,
    )
    LAST["res"] = res
    LAST["Ks"] = Ks

    # ---- host-side unshard ---------------------------------------------
    out_full = np.zeros((T, D_PROJ), dtype=np.float32)
    for k in range(N_CORES):
        o = np.asarray(res.results[k]["out"])
        rm = row_maps[k]
        valid = rm >= 0
        out_full[rm[valid]] = o[valid].astype(np.float32)
    return out_full.reshape(B, S, D_PROJ)


# revision 25
# speedup vs baseline: 1.1432x; 1.1432x over previous
"""Adaptive embedding lookup (nn.AdaptiveEmbedding) on 8 TRN2 NeuronCores.

Strategy (data-parallel over tokens, tables replicated, no collectives):

Host:
  - Bucket the 16384 tokens by embedding cluster (4 clusters; cluster 2 is
    split into 5 sub-ranges of 32000 rows so dma_gather's int16 indices stay
    in range), deal each bucket round-robin to the 8 cores, pad per-core
    buckets to a multiple of 128.
  - Clusters 0/1 (d_emb 1024/256) are FOLDED ON HOST: table' = emb @ proj.T
    * sqrt(d_proj) in f32, stored bf16 [20000, 1024].  On device these
    clusters are a pure gather -> output DMA (no matmul, no projection).
  - Cluster 2: table zero-padded to 128 bf16 cols (256B rows) so a
    TRANSPOSED gather lands rows as matmul-ready lhsT columns (partitions
    0..63 = features, 64..127 = zeros, never read).
  - Cluster 3: 8 rows packed per 128-col super-row; transposed gather +
    host-shipped 0/1 mask (selects the right 16-feature block per token,
    DVE multiply) + K=128 matmul against the 8x-replicated projection.

Device (SPMD, identical graph on all 8 cores, one TileContext):
  - The Q7 library holding dma_gather is preloaded as the first Pool-engine
    instruction so its ~10us IRAM load starts as early as possible.
  - idx/mask/proj loads on the two HWDGE queues; 8 dma_gathers spread over
    4 SWDGE queues (count registers hoisted -- one MOVE per distinct
    count); a long free-running burst of real warmup matmuls keeps the PE
    HAM clock warm until gathered data arrives (transposes don't count).
  - c0/c1: output rows DMA straight out of the gathered SBUF tiles.
  - c2: per 32000-row sub-range, 2 groups x 2 matmuls (K=64) -> PSUM ->
    bf16 copy -> one batched output DMA per sub-range.
  - c3: one DVE mask-multiply, then per group 2 matmuls (K=128) -> PSUM ->
    copy -> batched output DMA (last partial group trimmed).

Host: inverse-permute the 8 per-core outputs into [8, 2048, 1024] f32.
"""

import numpy as np
import ml_dtypes

import concourse.bacc as bacc
import concourse.bass as bass
import concourse.mybir as mybir
from concourse import library_config
from concourse.bass_utils import run_bass_kernel_spmd
from concourse.tile import TileContext

N_TOKEN = 267735
D_PROJ = 1024
CUTOFF_ENDS = [0, 20000, 40000, 200000, 267735]
EMB_SCALE = float(D_PROJ) ** 0.5
N_CORES = 8
P = 128
NFREE = 512          # psum free-dim per matmul
C2_SUB = 32000       # cluster-2 subtable rows (int16 range)
C2_NSUB = 5
C3_PACK = 8          # cluster-3 rows packed per super-row
C3_SROWS = -(-(CUTOFF_ENDS[4] - CUTOFF_ENDS[3]) // C3_PACK)  # 8467

BF16 = ml_dtypes.bfloat16

# Test-harness knobs (the grader never touches these).
TRACE = False
TRACE_CORES = None
LAST = {}
NWARM = 48

_GRAPH_CACHE = {}

# unit = gather bucket: 0, 1, (2, r) for sub-range r, 3.
UNIT_KEYS = [0, 1] + [(2, r) for r in range(C2_NSUB)] + [3]


def _build_graph(Ks, cnts):
    """Ks: unit -> group count; cnts: unit -> max used rows across cores."""
    key = tuple((Ks[u], cnts[u]) for u in UNIT_KEYS)
    if key in _GRAPH_CACHE:
        return _GRAPH_CACHE[key]

    K0, K1, K3 = Ks[0], Ks[1], Ks[3]
    K2 = sum(Ks[(2, r)] for r in range(C2_NSUB))
    G = sum(Ks.values())
    NI = 8 * G                      # idx16 columns (8 per group)

    nc = bacc.Bacc("TRN2", debug=False, num_swdge_queues=4,
                   enable_partition_id=False)
    idx_ext = nc.declare_dram_parameter("idx16", [P, max(NI, 16)], mybir.dt.int16, False)
    msk_ext = nc.declare_dram_parameter("msk", [P, max(K3, 1) * P], mybir.dt.bfloat16, False)
    emb0_ext = nc.declare_dram_parameter("emb0p", [20000, 1024], mybir.dt.bfloat16, False)
    emb1_ext = nc.declare_dram_parameter("emb1p", [20000, 1024], mybir.dt.bfloat16, False)
    emb2_ext = nc.declare_dram_parameter("emb2z", [160000, 128], mybir.dt.bfloat16, False)
    emb3_ext = nc.declare_dram_parameter("emb3p", [C3_SROWS, 128], mybir.dt.bfloat16, False)
    pt2_ext = nc.declare_dram_parameter("pt2", [64, D_PROJ], mybir.dt.bfloat16, False)
    pt3_ext = nc.declare_dram_parameter("pt3s", [P, D_PROJ], mybir.dt.bfloat16, False)
    out_ext = nc.declare_dram_parameter("out", [G * P, D_PROJ], mybir.dt.bfloat16, True)

    # idx16 column offset / group base / out-row base per unit
    unit_col, gbase = {}, {}
    col = acc = 0
    for u in UNIT_KEYS:
        unit_col[u] = col
        gbase[u] = acc
        col += 8 * Ks[u]
        acc += Ks[u]

    with TileContext(nc) as tc:
        with tc.tile_pool(name="const", bufs=1) as constp, \
             tc.tile_pool(name="ps_o", bufs=6, space="PSUM") as psump, \
             tc.tile_pool(name="ps_w", bufs=2, space="PSUM") as psumw:
            # start the ~10us Q7 IRAM load of the dma_gather library ASAP
            nc.gpsimd.load_library(library_config.mlp)

            idx_sb = constp.tile([P, max(NI, 16)], mybir.dt.int16, tag="idx")
            nc.sync.dma_start(out=idx_sb[:], in_=idx_ext[:])
            msk_sb = constp.tile([P, max(K3, 1) * P], mybir.dt.bfloat16, tag="msk")
            nc.scalar.dma_start(out=msk_sb[:], in_=msk_ext[:])
            pt2_sb = constp.tile([64, D_PROJ], mybir.dt.bfloat16, tag="pt2")
            nc.scalar.dma_start(out=pt2_sb[:], in_=pt2_ext[:])
            pt3_sb = constp.tile([P, D_PROJ], mybir.dt.bfloat16, tag="pt3")
            nc.scalar.dma_start(out=pt3_sb[:], in_=pt3_ext[:])

            junk = constp.tile([P, NFREE], mybir.dt.bfloat16, tag="junk")
            nc.vector.memset(junk[:], 0.0)

            e0 = constp.tile([P, max(K0, 1), 1024], mybir.dt.bfloat16, tag="e0")
            e1 = constp.tile([P, max(K1, 1), 1024], mybir.dt.bfloat16, tag="e1")
            e2T = constp.tile([P, max(K2, 1) * P], mybir.dt.bfloat16, tag="e2T")
            e3T = constp.tile([P, max(K3, 1) * P], mybir.dt.bfloat16, tag="e3T")
            em3 = constp.tile([P, max(K3, 1) * P], mybir.dt.bfloat16, tag="em3")
            osb2 = constp.tile([P, max(K2, 1), 1024], mybir.dt.bfloat16, tag="osb2")
            osb3 = constp.tile([P, max(K3, 1), 1024], mybir.dt.bfloat16, tag="osb3")

            # ---- gathers: c2 units first (they feed the PE critical path)
            j0_2 = {}
            jb = 0
            for r in range(C2_NSUB):
                j0_2[(2, r)] = jb
                jb += Ks[(2, r)]
            regs = {}
            for v in sorted({cnts[0], cnts[1]}
                            | {Ks[(2, r)] * P for r in range(C2_NSUB)}
                            | {Ks[3] * P}):
                regs[v] = nc.gpsimd.to_reg(v)
            gather_plan = [
                ((2, 0), 0), ((2, 1), 1), ((2, 2), 2), (3, 3),
                ((2, 3), 0), ((2, 4), 1), (0, 2), (1, 3),
            ]
            for u, q in gather_plan:
                n = Ks[u]
                if n == 0:
                    continue
                if u == 0:
                    dst, tab, elem, tr = e0[:], emb0_ext[:], 1024, False
                elif u == 1:
                    dst, tab, elem, tr = e1[:], emb1_ext[:], 1024, False
                elif u == 3:
                    dst = e3T[:, 0:n * P].rearrange("p (o i) -> p o i", o=1)
                    tab, elem, tr = emb3_ext[:], 128, True
                else:
                    r = u[1]
                    j0 = j0_2[u]
                    dst = e2T[:, j0 * P:(j0 + n) * P].rearrange(
                        "p (o i) -> p o i", o=1
                    )
                    tab = emb2_ext[r * C2_SUB:(r + 1) * C2_SUB, :]
                    elem, tr = 128, True
                c0 = unit_col[u]
                # non-transpose gathers skip trailing -1 indices; the runtime
                # count register must equal the number of valid indices (same
                # on every core -- host pads shorter cores with dummies)
                nreg = cnts[u] if u in (0, 1) else n * P
                nc.gpsimd.dma_gather(
                    dst, tab, idx_sb[:, c0:c0 + 8 * n], n * P, regs[nreg],
                    elem, transpose=tr, queue_num=q,
                )

            # ---- PE warmup: real matmuls engage the HAM clock ramp and
            # keep it warm until gathered data arrives (transposes don't)
            for w in range(NWARM):
                wps = psumw.tile([P, NFREE], mybir.dt.float32, tag="wps")
                nc.tensor.matmul(
                    out=wps[:], lhsT=junk[:, 0:P], rhs=junk[:],
                    start=True, stop=True,
                )

            # ---- output DMA helper: full groups in one rectangular DMA,
            # partial tail group separately (trimmed to used rows)
            dma_engs = [nc.sync, nc.scalar]
            qsel = [0]

            def emit_out(src3d, g0, gb, cnt):
                F = cnt // P
                rpart = cnt - F * P
                eng = dma_engs[qsel[0] % 2]
                qsel[0] += 1
                if F > 0:
                    dst = out_ext[gb * P:(gb + F) * P, :].rearrange(
                        "(g p) e -> p g e", p=P
                    )
                    eng.dma_start(out=dst, in_=src3d[:, g0:g0 + F, :])
                if rpart > 0:
                    eng2 = dma_engs[qsel[0] % 2]
                    qsel[0] += 1
                    eng2.dma_start(
                        out=out_ext[(gb + F) * P:(gb + F) * P + rpart, :],
                        in_=src3d[0:rpart, g0 + F:g0 + F + 1, :].rearrange(
                            "p o e -> p (o e)"
                        ),
                    )

            # ---- c0/c1: gathered rows ARE the output rows
            if K0 > 0:
                emit_out(e0, 0, gbase[0], cnts[0])
            if K1 > 0:
                emit_out(e1, 0, gbase[1], cnts[1])

            # ---- c2: per sub-range, matmuls then one batched output DMA
            def mm_group(lhsT, pt, psums):
                for oc, ps in enumerate(psums):
                    nc.tensor.matmul(
                        out=ps[:], lhsT=lhsT,
                        rhs=pt[:, oc * NFREE:(oc + 1) * NFREE],
                        start=True, stop=True,
                    )

            for r in range(C2_NSUB):
                u = (2, r)
                n = Ks[u]
                if n == 0:
                    continue
                j0 = j0_2[u]
                for j in range(n):
                    ps0 = psump.tile([P, NFREE], mybir.dt.float32, tag="ps")
                    ps1 = psump.tile([P, NFREE], mybir.dt.float32, tag="ps")
                    mm_group(e2T[0:64, (j0 + j) * P:(j0 + j + 1) * P],
                             pt2_sb, (ps0, ps1))
                    for oc, ps in enumerate((ps0, ps1)):
                        nc.any.tensor_copy(
                            out=osb2[:, j0 + j, oc * NFREE:(oc + 1) * NFREE],
                            in_=ps[:],
                        )
                rows = min(cnts[u], n * P)
                emit_out(osb2, j0, gbase[u],
                         n * P if n * P - rows <= 16 else rows)

            # ---- c3: mask-select then matmul (K=128 vs replicated proj)
            if K3 > 0:
                nc.vector.tensor_tensor(
                    out=em3[:, 0:K3 * P], in0=e3T[:, 0:K3 * P],
                    in1=msk_sb[:, 0:K3 * P], op=mybir.AluOpType.mult,
                )
                for j in range(K3):
                    ps0 = psump.tile([P, NFREE], mybir.dt.float32, tag="ps")
                    ps1 = psump.tile([P, NFREE], mybir.dt.float32, tag="ps")
                    mm_group(em3[:, j * P:(j + 1) * P], pt3_sb, (ps0, ps1))
                    for oc, ps in enumerate((ps0, ps1)):
                        nc.any.tensor_copy(
                            out=osb3[:, j, oc * NFREE:(oc + 1) * NFREE],
                            in_=ps[:],
                        )
                emit_out(osb3, 0, gbase[3], cnts[3])

    nc.compile()
    _GRAPH_CACHE[key] = nc
    return nc


def _wrap_idx16(vals, n_slots, fill=0):
    """int16 values (len <= n_slots, padded with fill) -> [128, n_slots/16]."""
    full = np.full(n_slots, fill, dtype=np.int16)
    full[:len(vals)] = vals
    w = np.zeros((16, n_slots // 16), dtype=np.int16)
    m = np.arange(n_slots)
    w[m % 16, m // 16] = full
    return np.tile(w, (8, 1))


def kernel(inp, emb0, emb1, emb2, emb3, proj0, proj1, proj2, proj3):
    inp = np.asarray(inp)
    embs = [np.asarray(e, dtype=np.float32) for e in (emb0, emb1, emb2, emb3)]
    projs = [np.asarray(p, dtype=np.float32) for p in (proj0, proj1, proj2, proj3)]
    B, S = inp.shape
    flat = inp.reshape(-1).astype(np.int64)
    T = flat.shape[0]

    # ---- host-side bucketing -------------------------------------------
    flat = np.clip(flat, 0, N_TOKEN - 1)
    cluster = np.clip(
        np.searchsorted(np.asarray(CUTOFF_ENDS[1:]), flat, side="right"), 0, 3
    )
    local = flat - np.asarray(CUTOFF_ENDS)[cluster]

    unit_pos = {}
    for u in UNIT_KEYS:
        if u == 0 or u == 1 or u == 3:
            unit_pos[u] = np.nonzero(cluster == u)[0]
        else:
            r = u[1]
            unit_pos[u] = np.nonzero((cluster == 2) & (local // C2_SUB == r))[0]

    core_lists = {u: [unit_pos[u][k::N_CORES] for k in range(N_CORES)]
                  for u in UNIT_KEYS}
    cnts = {u: max(len(core_lists[u][k]) for k in range(N_CORES))
            for u in UNIT_KEYS}
    Ks = {u: int(-(-max(cnts[u], 1) // P)) for u in UNIT_KEYS}
    G = sum(Ks.values())
    K3 = Ks[3]

    def idxval(u, positions):
        lv = local[positions]
        if u == 0 or u == 1:
            return lv.astype(np.int16)
        if u == 3:
            return (lv // C3_PACK).astype(np.int16)
        return (lv - u[1] * C2_SUB).astype(np.int16)

    NI = 8 * G
    gbase = {}
    acc = 0
    for u in UNIT_KEYS:
        gbase[u] = acc
        acc += Ks[u]

    blkid = np.arange(128) // 16

    idx_maps, msk_maps, row_maps = [], [], []
    for k in range(N_CORES):
        cols = []
        row_map = np.full(G * P, -1, dtype=np.int64)
        msk = np.zeros((P, max(K3, 1) * P), dtype=np.float32)
        for u in UNIT_KEYS:
            n = Ks[u]
            if n == 0:
                continue
            lst = core_lists[u][k]
            vals = idxval(u, lst)
            if u in (0, 1):
                # equalize valid-index count across cores (dummy repeats of
                # a valid index; their output rows are ignored), then -1-fill
                # the group-padding tail, which the gather engine skips
                if len(vals) < cnts[u]:
                    dummy = vals[0] if len(vals) else np.int16(0)
                    vals = np.concatenate(
                        [vals, np.full(cnts[u] - len(vals), dummy, np.int16)])
                cols.append(_wrap_idx16(vals, n * P, fill=-1))
            else:
                cols.append(_wrap_idx16(vals, n * P, fill=0))
            m = np.arange(len(lst))
            row_map[(gbase[u] + m // P) * P + (m % P)] = lst
            if u == 3:
                # transposed-gather layout: partition = element-in-super-row,
                # free column m = token m of this unit
                s_arr = local[lst] % C3_PACK
                msk[:, m] = (blkid[:, None] == s_arr[None, :])
        idx_host = (np.concatenate(cols, axis=1) if cols
                    else np.zeros((P, 16), np.int16))
        if idx_host.shape[1] < max(NI, 16):
            pad = np.zeros((P, max(NI, 16) - idx_host.shape[1]), np.int16)
            idx_host = np.concatenate([idx_host, pad], axis=1)
        idx_maps.append(np.ascontiguousarray(idx_host))
        msk_maps.append(np.ascontiguousarray(msk.astype(BF16)))
        row_maps.append(row_map)

    # ---- table/projection prep -----------------------------------------
    emb0p = np.ascontiguousarray(
        (embs[0] @ (projs[0].T * EMB_SCALE)).astype(BF16))
    emb1p = np.ascontiguousarray(
        (embs[1] @ (projs[1].T * EMB_SCALE)).astype(BF16))
    emb2z = np.zeros((160000, 128), dtype=BF16)
    emb2z[:, :64] = embs[2].astype(BF16)
    emb2z = np.ascontiguousarray(emb2z)
    e3flat = embs[3]
    pad3 = C3_SROWS * C3_PACK - e3flat.shape[0]
    e3flat = np.concatenate([e3flat, np.zeros((pad3, 16), np.float32)], axis=0)
    emb3p = np.ascontiguousarray(e3flat.reshape(C3_SROWS, 128).astype(BF16))

    pt2 = np.ascontiguousarray((projs[2].T * EMB_SCALE).astype(BF16))
    pt3 = projs[3].T * EMB_SCALE
    pt3s = np.ascontiguousarray(np.tile(pt3, (C3_PACK, 1)).astype(BF16))

    in_maps = []
    for k in range(N_CORES):
        in_maps.append({
            "idx16": idx_maps[k], "msk": msk_maps[k],
            "emb0p": emb0p, "emb1p": emb1p, "emb2z": emb2z, "emb3p": emb3p,
            "pt2": pt2, "pt3s": pt3s,
        })

    # ---- device --------------------------------------------------------
    nc = _build_graph(Ks, cnts)
    res = run_bass_kernel_spmd(
        nc,
        in_maps,
        core_ids=list(range(N_CORES)),
        trace=TRACE,
        trace_cores=TRACE_CORES,
    )
    LAST["res"] = res
    LAST["Ks"] = Ks

    # ---- host-side unshard ---------------------------------------------
    out_full = np.zeros((T, D_PROJ), dtype=np.float32)
    for k in range(N_CORES):
        o = np.asarray(res.results[k]["out"])
        rm = row_maps[k]
        valid = rm >= 0
        out_full[rm[valid]] = o[valid].astype(np.float32)
    return out_full.reshape(B, S, D_PROJ)
